# revision 1
# baseline (speedup 1.0000x reference)
"""Trainium2 Bass kernel for nn_MixtureOfTokensLayer.

Math (per sequence position s; B=32 tokens form ONE group of k=32):
  logits = x_s @ controller            (32, 8)
  w      = softmax_k(logits)           (32, 8)
  merged = w.T @ x_s                   (8, 1024)   per-expert token mix
  h      = relu(merged_e @ lin1_e)     (8, 512)
  eo     = h_e @ lin2_e                (8, 1024)
  out_s  = w @ eo                      (32, 1024)

Sharding: data-parallel over S (2048 -> 256 per core, 8 cores). No
collectives. All compute in bf16 with fp32 PSUM accumulation; output
returned to host as bf16 and upcast (error stays ~6e-3 rel).

Per-core dataflow (block = 4 s-positions = 128 tokens on partitions),
pipelined over 2 s-chunks of 128 s; chunk B's P1 (input DMA + routing) is
source-interleaved with chunk A's P2b/P3 (MLP tail + emit + output DMA)
so input and output DMA overlap:
  P1 per 16-s tile: logits^T = ctl.T @ xT (PE) -> PE-transpose 128-token
     chunks -> exp (ACT, no max-subtraction: logits are O(5)) -> build
     block-diagonal wbd (128 tok x 32 (s,e)) with one masked multiply ->
     merge MM lhsT=wbd gives merged with (s,e) ON PARTITIONS; Z = wbd.T @
     ones (same lhsT); 1/Z folded into the PSUM evacuations (relu
     positive-homogeneity lets normalization commute); PE-transpose
     merged -> merged^T (d-major); PE-transpose wbd -> wbdT (emit lhsT).
  P2a: lin1 per expert (w1 streamed per chunk, ACT DGE ring).
  P2b + P3 per d-half: lin2 (w2 resident); PE-transpose eo^T ->
     (s,e)-row blocks; emit MM out = wbdT_scaled.T @ eo_block; evac bf16
     (DVE/ACT alternating); DMA out on the ACT DGE ring.
"""

import os
import sys

import numpy as np
import ml_dtypes

sys.path.insert(0, "/opt/trn_rl_repo")

import concourse.bass as bass
import concourse.mybir as mybir
import concourse.tile as tile
from concourse import bacc

B, S, D, F, E, K = 32, 2048, 1024, 512, 8, 32
N_CORES = 8
TILE_S = 16              # s-positions per P1/P3 tile
NBLK = 4                 # blocks per tile
TOK = 128                # tokens per block (4 s * 32 k)
DC = D // 128            # 8 d-chunks
FC = F // 128            # 4 f-chunks
CHUNK_S = 128            # s-positions per pipeline chunk
BF = mybir.dt.bfloat16
F32 = mybir.dt.float32
AF = mybir.ActivationFunctionType


class _Ctx:
    pass


def moe_body(tc, xg, xT, w1, w2, ctl, idf, idb, msk, out, s_local, reps=1):
    nc = tc.nc
    import contextlib

    with (
        tc.tile_pool(name="const", bufs=1) as const,
        tc.tile_pool(name="resw", bufs=1) as resw,
        tc.tile_pool(name="persist", bufs=2) as persist,
        tc.tile_pool(name="pers1", bufs=1) as pers1,
        tc.tile_pool(name="xstream", bufs=3) as xsp,
        tc.tile_pool(name="wstream", bufs=2) as wsp,
        tc.tile_pool(name="p1", bufs=2) as p1,
        tc.tile_pool(name="outst", bufs=4) as outst,
        tc.tile_pool(name="ps_bigA", bufs=2, space="PSUM") as ps_big,
        tc.tile_pool(name="ps_bigB", bufs=2, space="PSUM") as ps_out,
        tc.tile_pool(name="ps_sm", bufs=4, space="PSUM") as ps_sm,
    ):
        c = _Ctx()
        c.tc, c.nc = tc, nc
        c.xsp, c.wsp, c.p1, c.persist, c.pers1, c.outst = (
            xsp, wsp, p1, persist, pers1, outst)
        c.ps_big, c.ps_out, c.ps_sm = ps_big, ps_out, ps_sm
        c.w1, c.w2 = w1, w2

        # ---- constants ----
        c.ctl_sb = const.tile([128, DC, E], BF, name="ctl_sb")
        nc.sync.dma_start(c.ctl_sb[:], ctl)
        c.idf_sb = const.tile([128, 128], F32, name="idf_sb")
        nc.sync.dma_start(c.idf_sb[:], idf)
        c.idb_sb = const.tile([128, 128], BF, name="idb_sb")
        nc.sync.dma_start(c.idb_sb[:], idb)
        c.msk_sb = const.tile([128, NBLK, E], BF, name="msk_sb")
        nc.sync.dma_start(c.msk_sb[:], msk)
        c.ones_sb = const.tile([128, 1], BF, name="ones_sb")
        nc.vector.memset(c.ones_sb[:], 1.0)
        # resident lin2 (read once; per-dt so x loads interleave)
        c.w2_sb = resw.tile([128, DC, FC, E, 128], BF, name="w2_sb")
        for dt in range(DC):
            nc.scalar.dma_start(c.w2_sb[:, dt], w2[:, dt])

        c.xg_r = xg.rearrange("(nb p) d -> p nb d", p=128)
        c.xT_r = xT.rearrange("(dc p) t -> p dc t", p=128)
        c.out_r = out.rearrange("(nb p) d -> p nb d", p=128)

        n_chunks = max(1, s_local // CHUNK_S)
        chunk_s = s_local // n_chunks
        from itertools import chain as _ichain

        phases = "123"
        rep_ctx = tc.For_i(0, reps, 1) if reps > 1 else contextlib.nullcontext()
        with rep_ctx:
            prev_p23 = None
            for chunk in range(n_chunks):
                st = _chunk_state(c, chunk, chunk_s)
                gen = _p1_steps(c, st)
                if prev_p23 is None:
                    _drain(gen)
                else:
                    # interleave prev chunk's P2b/P3 with this chunk's P1
                    # (P2a must NOT join this interleave: its h_ps shares the
                    # "big" psum tag with prev P23's eo_ps -> slot deadlock)
                    _interleave(prev_p23, gen, ratio=3)
                if "2" in phases:
                    _drain(_p2a_steps(c, st))
                if "3" in phases:
                    prev_p23 = _p23_steps(c, st)
            if prev_p23 is not None:
                _drain(prev_p23)


def _drain(g):
    for _ in g:
        pass


def _interleave(g_slow, g_fast, ratio=2):
    """Alternate emitting `ratio` steps of g_slow per step of g_fast."""
    done_s = done_f = False
    while not (done_s and done_f):
        for _ in range(ratio):
            if not done_s:
                done_s = next(g_slow, _SENTINEL) is _SENTINEL
        if not done_f:
            done_f = next(g_fast, _SENTINEL) is _SENTINEL


_SENTINEL = object()


def _chunk_state(c, chunk, chunk_s):
    st = _Ctx()
    st.chunk = chunk
    st.cs = chunk_s
    st.n_tiles = chunk_s // TILE_S
    st.tile0 = chunk * st.n_tiles
    st.mT = c.persist.tile([128, DC, E, chunk_s], BF, tag="mT",
                           name=f"mT{chunk}")
    st.wbdT = c.persist.tile([128, st.n_tiles, TOK], BF, tag="wbdT",
                             name=f"wbdT{chunk}")
    st.h_all = c.pers1.tile([128, FC, E, chunk_s], BF, tag="h",
                            name=f"h{chunk}")
    st.eoT = c.pers1.tile([128, DC, chunk_s, E], BF, tag="eoT",
                          name=f"eoT{chunk}")
    return st


def _p1_steps(c, st):
    nc = c.nc
    for t in range(st.n_tiles):
        ti = st.tile0 + t
        xT_t = c.xsp.tile([128, DC, TILE_S * K], BF, tag="xT", name=f"xT{ti}")
        nc.sync.dma_start(xT_t[:], c.xT_r[:, :, ti * 512:(ti + 1) * 512])
        xg_t = c.xsp.tile([128, NBLK, D], BF, tag="xg", name=f"xg{ti}")
        nc.sync.dma_start(xg_t[:], c.xg_r[:, ti * NBLK:(ti + 1) * NBLK, :])

        # logits^T (e, 512 tokens), accumulate over d-chunks
        lg_ps = c.ps_sm.tile([8, 512], F32, tag="sm", name=f"lgps{ti}")
        for dc in range(DC):
            nc.tensor.matmul(lg_ps[:], c.ctl_sb[:, dc, :], xT_t[:, dc, :],
                             start=(dc == 0), stop=(dc == DC - 1))
        lgT = c.p1.tile([8, 512], F32, tag="lgT", name=f"lgT{ti}")
        nc.vector.tensor_copy(out=lgT[:], in_=lg_ps[:])

        # per block: transpose 128-token chunk; one exp for all 4 blocks
        expl = c.p1.tile([128, NBLK, E], BF, tag="expl", name=f"expl{ti}")
        tp_ps = c.ps_sm.tile([128, NBLK, 8], F32, tag="sm", name=f"tpps{ti}")
        for b in range(NBLK):
            nc.tensor.transpose(tp_ps[:, b, :], lgT[:, b * TOK:(b + 1) * TOK],
                                c.idf_sb[:8, :8])
        nc.scalar.activation(expl[:], tp_ps[:], AF.Exp)

        mg_ps = [c.ps_big.tile([128, 512], F32, tag="big", name=f"mgps{ti}{h}")
                 for h in range(2)]
        z_ps = c.ps_sm.tile([128, 1], F32, tag="sm", name=f"zps{ti}")
        wt_ps = c.ps_sm.tile([128, TOK], BF, tag="sm", name=f"wtps{ti}")
        for b in range(NBLK):
            # block-diag wbd (128 tok, 4s x 8e) via one masked multiply
            wbd = c.p1.tile([TOK, NBLK, E], BF, tag="wbd", name=f"wbd{ti}_{b}")
            nc.vector.tensor_tensor(
                out=wbd[:],
                in0=expl[:, b, None, :].to_broadcast((TOK, NBLK, E)),
                in1=c.msk_sb[:],
                op=mybir.AluOpType.mult)
            r0 = 32 * b
            nc.tensor.matmul(z_ps[r0:r0 + 32, :], wbd[:], c.ones_sb[:],
                             start=True, stop=True, tile_position=(0, r0))
            nc.tensor.matmul(mg_ps[0][r0:r0 + 32, :], wbd[:],
                             xg_t[:, b, 0:512], start=True, stop=True,
                             tile_position=(0, r0))
            nc.tensor.matmul(mg_ps[1][r0:r0 + 32, :], wbd[:],
                             xg_t[:, b, 512:1024], start=True, stop=True,
                             tile_position=(0, r0))
            nc.tensor.transpose(wt_ps[r0:r0 + 32, :], wbd[:], c.idb_sb[:],
                                tile_position=(0, r0))

        zr = c.p1.tile([128, 1], F32, tag="zr", name=f"zr{ti}")
        nc.vector.reciprocal(zr[:], z_ps[:])
        # evacuations with 1/Z folded in (per-partition scalar = per (s,e))
        mg_sb = c.p1.tile([128, D], BF, tag="mgsb", name=f"mgsb{ti}")
        nc.vector.tensor_scalar_mul(mg_sb[:, 0:512], mg_ps[0][:], zr[:])
        nc.vector.tensor_scalar_mul(mg_sb[:, 512:1024], mg_ps[1][:], zr[:])
        nc.vector.tensor_scalar_mul(st.wbdT[:, t, :], wt_ps[:], zr[:])

        # merged -> merged^T (d on partitions); 4 transposes per evac
        for g in range(2):
            mt_ps = c.ps_sm.tile([128, 4, 128], BF, tag="sm",
                                 name=f"mtps{ti}_{g}")
            for j in range(4):
                dc = g * 4 + j
                nc.tensor.transpose(mt_ps[:, j, :],
                                    mg_sb[:, dc * 128:(dc + 1) * 128],
                                    c.idb_sb[:])
            nc.vector.tensor_copy(
                out=st.mT[:, g * 4:(g + 1) * 4, :,
                          t * TILE_S:(t + 1) * TILE_S],
                in_=mt_ps[:].rearrange("p j (s e) -> p j e s", e=E))
        yield


def _p2a_steps(c, st):
    nc = c.nc
    for e in range(E):
        w1_t = c.wsp.tile([128, DC, F], BF, tag="w1", name=f"w1_{st.chunk}_{e}")
        nc.scalar.dma_start(w1_t[:], c.w1[:, :, e, :])
        h_ps = c.ps_big.tile([128, FC, st.cs], F32, tag="big",
                             name=f"hps{st.chunk}_{e}")
        for ft in range(FC):
            for dc in range(DC):
                nc.tensor.matmul(h_ps[:, ft, :],
                                 w1_t[:, dc, ft * 128:(ft + 1) * 128],
                                 st.mT[:, dc, e, :],
                                 start=(dc == 0), stop=(dc == DC - 1))
        nc.scalar.activation(st.h_all[:, :, e, :], h_ps[:], AF.Relu)
        yield


def _p23_steps(c, st):
    nc = c.nc
    for half in range(2):
        for e in range(E):
            # 4 dt-planes (one d-half) share one psum bank; evac reads
            # contiguous psum, writes eoT strided (s-stride E)
            eo_ps = c.ps_big.tile([128, 4, st.cs], F32, tag="big",
                                  name=f"eops{st.chunk}_{half}_{e}")
            for j in range(4):
                dt = half * 4 + j
                for fc in range(FC):
                    nc.tensor.matmul(eo_ps[:, j, :],
                                     c.w2_sb[:, dt, fc, e, :],
                                     st.h_all[:, fc, e, :],
                                     start=(fc == 0), stop=(fc == FC - 1))
            ev = nc.scalar.copy if (half + e) % 2 else nc.vector.tensor_copy
            ev(out=st.eoT[:, half * 4:(half + 1) * 4, :, e], in_=eo_ps[:])
            yield

        # emit for this d-half
        for t in range(st.n_tiles):
            ti = st.tile0 + t
            eo_blk = c.p1.tile([128, 4, 128], BF, tag="eoblk",
                               name=f"eoblk{half}_{ti}")
            et_ps = c.ps_sm.tile([128, 4, 128], BF, tag="sm",
                                 name=f"etps{half}_{ti}")
            for j in range(4):
                dt = half * 4 + j
                src = st.eoT[:, dt, t * TILE_S:(t + 1) * TILE_S, :]
                nc.tensor.transpose(et_ps[:, j, :],
                                    src.rearrange("p s e -> p (s e)"),
                                    c.idb_sb[:])
            ev = nc.scalar.copy if t % 2 else nc.vector.tensor_copy
            ev(out=eo_blk[:], in_=et_ps[:])
            for b in range(NBLK):
                o_ps = c.ps_out.tile([128, 512], F32, tag="out",
                                     name=f"ops{half}_{ti}_{b}")
                r0 = 32 * b
                nc.tensor.matmul(o_ps[:], st.wbdT[r0:r0 + 32, t, :],
                                 eo_blk[r0:r0 + 32, :, :],
                                 start=True, stop=True, tile_position=(r0, 0))
                o_sb = c.outst.tile([128, 512], BF, tag="osb",
                                    name=f"osb{half}_{ti}_{b}")
                if b % 2 == 0:
                    nc.vector.tensor_copy(out=o_sb[:], in_=o_ps[:])
                else:
                    nc.scalar.copy(out=o_sb[:], in_=o_ps[:])
                nc.scalar.dma_start(
                    c.out_r[:, ti * NBLK + b, half * 512:(half + 1) * 512],
                    o_sb[:])
            yield


def build_module(s_local, num_devices, reps=1):
    T = s_local * K
    nc = bacc.Bacc("TRN2", target_bir_lowering=False, debug=False,
                   num_devices=num_devices)
    xg = nc.dram_tensor("xg", [T, D], BF, kind="ExternalInput").ap()
    xT = nc.dram_tensor("xT", [D, T], BF, kind="ExternalInput").ap()
    w1 = nc.dram_tensor("w1", [128, DC, E, F], BF, kind="ExternalInput").ap()
    w2 = nc.dram_tensor("w2", [128, DC, FC, E, 128], BF,
                        kind="ExternalInput").ap()
    ctl = nc.dram_tensor("ctl", [128, DC, E], BF, kind="ExternalInput").ap()
    idf = nc.dram_tensor("idf", [128, 128], F32, kind="ExternalInput").ap()
    idb = nc.dram_tensor("idb", [128, 128], BF, kind="ExternalInput").ap()
    msk = nc.dram_tensor("msk", [128, NBLK, E], BF, kind="ExternalInput").ap()
    out = nc.dram_tensor("out", [T, D], BF, kind="ExternalOutput").ap()
    with tile.TileContext(nc) as tc:
        moe_body(tc, xg, xT, w1, w2, ctl, idf, idb, msk, out, s_local,
                 reps=reps)
    nc.compile()
    return nc


def stage_weights(lin1, lin2, controller):
    bf = ml_dtypes.bfloat16
    w1h = np.ascontiguousarray(
        lin1.reshape(E, DC, 128, F).transpose(2, 1, 0, 3)).astype(bf)
    # (128p, dt, fc, e, 128c): element = lin2[e, fc*128+p, dt*128+c]
    w2h = np.ascontiguousarray(
        lin2.reshape(E, FC, 128, DC, 128).transpose(2, 3, 1, 0, 4)).astype(bf)
    ctlh = np.ascontiguousarray(
        controller.reshape(DC, 128, E).transpose(1, 0, 2)).astype(bf)
    return w1h, w2h, ctlh


def stage_consts():
    bf = ml_dtypes.bfloat16
    idf = np.eye(128, dtype=np.float32)
    idb = np.eye(128, dtype=bf)
    msk = np.zeros((128, NBLK, E), np.float32)
    for st in range(NBLK):
        msk[st * K:(st + 1) * K, st, :] = 1.0
    return idf, idb, msk.astype(bf)


def stage_x(xs):
    """xs: (B, s_local, D) fp32 -> (xg bf16 (T, D), xT bf16 (D, T))."""
    s_local = xs.shape[1]
    bf = ml_dtypes.bfloat16
    xg_h = np.ascontiguousarray(
        xs.transpose(1, 0, 2).reshape(s_local * K, D)).astype(bf)
    xT_h = np.ascontiguousarray(xg_h.T)
    return xg_h, xT_h


_MODULE_CACHE = {}


def kernel(x, lin1, lin2, controller):
    from concourse.bass_utils import run_bass_kernel_spmd

    s_local = S // N_CORES
    key = (s_local, N_CORES)
    if key not in _MODULE_CACHE:
        _MODULE_CACHE[key] = build_module(s_local, N_CORES)
    nc = _MODULE_CACHE[key]

    w1h, w2h, ctlh = stage_weights(lin1, lin2, controller)
    idf, idb, msk = stage_consts()
    in_maps = []
    for c in range(N_CORES):
        xg_h, xT_h = stage_x(x[:, c * s_local:(c + 1) * s_local, :])
        in_maps.append({"xg": xg_h, "xT": xT_h, "w1": w1h, "w2": w2h,
                        "ctl": ctlh, "idf": idf, "idb": idb, "msk": msk})

    res = run_bass_kernel_spmd(nc, in_maps, core_ids=list(range(N_CORES)))
    out_full = np.empty((B, S, D), np.float32)
    for c in range(N_CORES):
        oc = np.asarray(res.results[c]["out"]).astype(np.float32)
        out_full[:, c * s_local:(c + 1) * s_local, :] = (
            oc.reshape(s_local, K, D).transpose(1, 0, 2))
    kernel.last_results = res
    return out_full



# revision 41
# speedup vs baseline: 1.8467x; 1.8467x over previous
"""Trainium2 Bass kernel for nn_MixtureOfTokensLayer.

Math (per sequence position s; B=32 tokens form ONE group of k=32):
  logits = x_s @ controller            (32, 8)
  w      = softmax_k(logits)           (32, 8)
  merged = w.T @ x_s                   (8, 1024)   per-expert token mix
  h      = relu(merged_e @ lin1_e)     (8, 512)
  eo     = h_e @ lin2_e                (8, 1024)
  out_s  = w @ eo                      (32, 1024)

Sharding: data-parallel over S (2048 -> 256 per core, 8 cores), no
collectives. All compute bf16 with fp32 PSUM; output bf16, host upcasts.

Dataflow (single s-chunk of 256; x staged HOST-SIDE as x^T only — the
tok-major copy is rebuilt on-chip by PE transposes, saving 16.8 MB of
HBM traffic per core):
  P1 per 16-s tile (512 tokens, 4 blocks of 128), 1-tile A/B skew so the
  PE never waits on the DVE softmax chain:
    A(t): PE-transpose x^T blocks -> xg blocks (bf16 psum, evac split
      DVE/ACT); logits via narrow matmuls (lhsT = x^T block, rhs = ctl
      chunk, out [tok, 8] accumulated over d-chunks); exp (ACT);
      per-token Z via block-diag-ones matmul; then DVE: 1/Z, normalized
      weights, block-diag wbd.
    B(t-1): merged^T directly (lhsT = xg block chunk, rhs = wbd, out
      [d-chunk, 32] — no merge transposes); wbdT via PE transpose.
  P2 per expert: lin1 (w1 chunk stationary, stream merged^T, N=256),
    relu evac -> h (f-major).
  P3 per d-half: lin2 per expert (w2 stationary, stream h, N=256) ->
    eoT (d-major); then per tile: PE-transpose eoT -> (s,e)-major
    blocks; emit matmuls (lhsT = wbdT row blocks, row-tiled); evac bf16
    alternating DVE/ACT; one output DMA per (half, tile).

Weight DMAs are issued from the sync (SP) ring, interleaved between x
tile loads so transfers land just before their consumers.
"""

import sys

import numpy as np
import ml_dtypes

sys.path.insert(0, "/opt/trn_rl_repo")

import concourse.bass as bass
import concourse.mybir as mybir
import concourse.tile as tile
from concourse import bacc

B, S, D, F, E, K = 32, 2048, 1024, 512, 8, 32
N_CORES = 8
TILE_S = 16              # s-positions per P1 tile
NBLK = 4                 # 128-token blocks per tile
TOK = 128                # tokens per block (4 s * 32 k)
DC = D // 128            # 8 d-chunks
FC = F // 128            # 4 f-chunks
BF = mybir.dt.bfloat16
F32 = mybir.dt.float32
AF = mybir.ActivationFunctionType


class _Ctx:
    pass


def moe_body(tc, xT, w1, w2, ctl, idb, abd, msk, out, s_local, reps=1):
    nc = tc.nc
    import contextlib

    n_tiles = s_local // TILE_S
    cs = s_local

    with (
        tc.tile_pool(name="const", bufs=1) as const,
        tc.tile_pool(name="pers", bufs=1) as pers,
        tc.tile_pool(name="xstream", bufs=3) as xsp,
        tc.tile_pool(name="xgp", bufs=2) as xgp,
        tc.tile_pool(name="route", bufs=2) as rte,
        tc.tile_pool(name="wstream", bufs=2) as wsp,
        tc.tile_pool(name="outst", bufs=3) as outst,
        tc.tile_pool(name="psA", bufs=3, space="PSUM") as psA,
        tc.tile_pool(name="psB", bufs=3, space="PSUM") as psB,
        tc.tile_pool(name="psC", bufs=2, space="PSUM") as psC,
    ):
        c = _Ctx()
        c.tc, c.nc = tc, nc
        c.xsp, c.xgp, c.rte, c.wsp, c.outst = xsp, xgp, rte, wsp, outst
        c.psA, c.psB, c.psC = psA, psB, psC
        c.w1, c.w2 = w1, w2
        c.n_tiles, c.cs = n_tiles, cs

        # ---- constants ----
        c.ctl_sb = const.tile([128, DC, E], BF, name="ctl_sb")
        nc.sync.dma_start(c.ctl_sb[:], ctl)
        c.idb_sb = const.tile([128, 128], BF, name="idb_sb")
        nc.sync.dma_start(c.idb_sb[:], idb)
        c.abd_sb = const.tile([128, 128], BF, name="abd_sb")
        nc.sync.dma_start(c.abd_sb[:], abd)
        c.msk_sb = const.tile([128, NBLK, E], BF, name="msk_sb")
        nc.sync.dma_start(c.msk_sb[:], msk)

        # ---- persistent intermediates ----
        # merged^T, (s,e)-interleaved columns: [d-part, dc, s, e].
        # eoT ([d-part, dt, s, e]) aliases the same slot later: mT is dead
        # once lin1 has consumed it, so tag-generation rotation reuses it.
        c.pers = pers
        c.mT = pers.tile([128, DC, cs, E], BF, tag="big", bufs=1, name="mT")
        # h, f-major: [f-part, fc, s, e]
        c.h_all = pers.tile([128, FC, cs, E], BF, name="h_all")
        # emit stationary: [(s,e)-part, tile, tok]
        c.wbdT = pers.tile([128, n_tiles, TOK], BF, name="wbdT")

        c.xT_r = xT.rearrange("(dc p) t -> p dc t", p=128)
        c.out_r = out.rearrange("(nb p) d -> p nb d", p=128)

        # w1/w2 stream tiles, DMA'd from the sync ring interleaved with x
        c.w1t = [None] * E
        c.w2t = [None] * (2 * E)

        W1_AT = [50 + 2.9 * i for i in range(E)]
        W2_AT = [72 + 1.5 * i for i in range(2 * E)]

        def load_w1(e, eng=None):
            # hold weight transfers behind the latency-critical x stream
            with tc.tile_wait_until(W1_AT[e] / 1000):
                c.w1t[e] = c.wsp.tile([128, DC, F], BF, tag="w1",
                                      bufs=7, name=f"w1_{e}")
                (eng or nc.sync).dma_start(c.w1t[e][:], c.w1[:, e])

        def load_w2(i):
            half, we = i // E, i % E
            with tc.tile_wait_until(W2_AT[i] / 1000):
                c.w2t[i] = c.wsp.tile([128, 4, FC, 128], BF, tag="w2",
                                      bufs=10, name=f"w2_{half}_{we}")
                nc.sync.dma_start(c.w2t[i][:],
                                  c.w2[:, we, half * 4:(half + 1) * 4])

        rep_ctx = tc.For_i(0, reps, 1) if reps > 1 else contextlib.nullcontext()
        with rep_ctx:
            prev = None  # A-phase state for the 1-tile skew
            for t in range(n_tiles):
                stt = _p1_partA(c, t)
                if prev is not None:
                    _p1_partB(c, prev)
                prev = stt
                # first w1 load rides the very tail of the x stream
                if t == 15:
                    load_w1(0, eng=nc.sync)
            _p1_partB(c, prev)
            # rest of the weights queue behind x on the idle sync ring;
            # pool slot rotation throttles them to just-in-time
            for e in range(1, E):
                load_w1(e)
            for i in range(2 * E):
                load_w2(i)
            for e in range(E):
                _p2_expert(c, e)
            # expert outs, d-major: [d-part, dt, s, e] — reuses mT's slot
            c.eoT = c.pers.tile([128, DC, cs, E], BF, tag="big", bufs=1,
                                name="eoT")
            # lin2 runs in s-halves (s-outer) so each emit batch of 8
            # tiles unblocks after only half the lin2 work of its d-half
            for e in range(E):
                _p3_lin2(c, 0, e, 0)
            gens = [_emit_gen(c, 0, 0, 8), _emit_gen(c, 0, 8, 16),
                    _emit_gen(c, 1, 0, 8)]
            sections = [(0, 1), (1, 0), (1, 1)]
            for g, (half, sh) in zip(gens, sections):
                for e in range(E):
                    _p3_lin2(c, half, e, sh)
                    if e % 2:
                        next(g, None)
            for g in gens:
                for _ in g:
                    pass
            for _ in _emit_gen(c, 1, 8, 16):
                pass


def _p1_partA(c, t):
    nc = c.nc
    xt = c.xsp.tile([128, DC, TILE_S * K], BF, tag="xT", name=f"xT{t}")
    nc.sync.dma_start(xt[:], c.xT_r[:, :, t * 512:(t + 1) * 512])

    xg = c.xgp.tile([128, NBLK, D], BF, tag="xg", name=f"xg{t}")
    expl = c.rte.tile([128, NBLK, E], BF, tag="expl", name=f"expl{t}")
    # one logits psum tile per 16-s tile: the four per-block accumulation
    # groups run sequentially in the same bank
    lg_ps = c.psB.tile([128, NBLK, E], F32, tag="B", name=f"lgps{t}")
    for b in range(NBLK):
        blk = xt[:, :, b * TOK:(b + 1) * TOK]
        # transposes: x^T block -> xg block (bf16 psum)
        xg_ps = c.psA.tile([128, DC, TOK], BF, tag="A", name=f"xgps{t}_{b}")
        for dc in range(DC):
            nc.tensor.transpose(xg_ps[:, dc, :], blk[:, dc, :], c.idb_sb[:])
        # logits: [tok, 8] accumulated over d-chunks
        for dc in range(DC):
            nc.tensor.matmul(lg_ps[:, b, :], blk[:, dc, :],
                             c.ctl_sb[:, dc, :],
                             start=(dc == 0), stop=(dc == DC - 1))
        if b % 4 == 3:
            ev = _act_copy(nc)
        else:
            ev = nc.vector.tensor_copy
        ev(out=xg[:, b, :], in_=xg_ps[:].rearrange("p dc k -> p (dc k)"))
    nc.scalar.activation(expl[:], lg_ps[:], AF.Exp)

    # per-token Z (replicated over the 32 tokens of each s)
    z_ps = c.psB.tile([128, NBLK, E], F32, tag="B", name=f"zps{t}")
    for b in range(NBLK):
        nc.tensor.matmul(z_ps[:, b, :], c.abd_sb[:], expl[:, b, :],
                         start=True, stop=True)
    zr = c.rte.tile([128, NBLK, E], F32, tag="zr", name=f"zr{t}")
    nc.vector.reciprocal(zr[:], z_ps[:])
    expn = c.rte.tile([128, NBLK, E], BF, tag="expn", name=f"expn{t}")
    nc.vector.tensor_tensor(out=expn[:], in0=expl[:], in1=zr[:],
                            op=mybir.AluOpType.mult)
    wbd = c.rte.tile([TOK, NBLK, NBLK * E], BF, tag="wbd", name=f"wbd{t}")
    for b in range(NBLK):
        nc.vector.tensor_tensor(
            out=wbd[:, b, :],
            in0=expn[:, b, None, :].to_broadcast((TOK, NBLK, E)),
            in1=c.msk_sb[:],
            op=mybir.AluOpType.mult)

    stt = _Ctx()
    stt.t, stt.xg, stt.wbd = t, xg, wbd
    return stt


def _p1_partB(c, stt):
    nc = c.nc
    t, xg, wbd = stt.t, stt.xg, stt.wbd
    wt_ps = c.psC.tile([128, TOK], BF, tag="Cmt", name=f"wtps{t}")
    for p in range(2):  # block pairs share a 2-bank psum tile
        mt_ps = c.psC.tile([128, 2, DC, NBLK * E], F32, tag="Cmt",
                           name=f"mtps{t}_{p}")
        for w in range(2):
            b = p * 2 + w
            # merged^T: lhsT = xg block chunk, rhs = wbd -> [d-chunk, 32]
            for dc in range(DC):
                nc.tensor.matmul(mt_ps[:, w, dc, :],
                                 xg[:, b, dc * 128:(dc + 1) * 128],
                                 wbd[:, b, :], start=True, stop=True)
            r0 = 32 * b
            nc.tensor.transpose(wt_ps[r0:r0 + 32, :], wbd[:, b, :],
                                c.idb_sb[:], tile_position=(0, r0))
        s0 = t * TILE_S + p * 2 * NBLK
        dst = c.mT[:, :, s0:s0 + 2 * NBLK, :]
        nc.scalar.copy(
            out=dst.rearrange("p dc (w s) e -> p w dc s e", w=2),
            in_=mt_ps[:].rearrange("p w dc (s e) -> p w dc s e", e=E))
    nc.scalar.copy(out=c.wbdT[:, t, :], in_=wt_ps[:])


def _act_copy(nc):
    def f(out, in_):
        return nc.scalar.copy(out=out, in_=in_)
    return f


def _p2_expert(c, e):
    nc = c.nc
    w1t = c.w1t[e]
    for fp in range(2):  # ft pairs
        h_ps = c.psA.tile([128, 2, c.cs], F32, tag="A", name=f"hps{e}_{fp}")
        for j in range(2):
            ft = fp * 2 + j
            for dc in range(DC):
                nc.tensor.matmul(h_ps[:, j, :],
                                 w1t[:, dc, ft * 128:(ft + 1) * 128],
                                 c.mT[:, dc, :, e],
                                 start=(dc == 0), stop=(dc == DC - 1))
        nc.scalar.activation(c.h_all[:, fp * 2:(fp + 1) * 2, :, e], h_ps[:],
                             AF.Relu)


def _p3_lin2(c, half, e, sh):
    nc = c.nc
    w2t = c.w2t[half * E + e]
    hs = c.cs // 2
    s0 = sh * hs
    for dp in range(2):  # dt pairs within the half
        eo_ps = c.psA.tile([128, 2, hs], F32, tag="A",
                           name=f"eops{half}_{e}_{sh}_{dp}")
        for j in range(2):
            dt = dp * 2 + j
            for fc in range(FC):
                nc.tensor.matmul(eo_ps[:, j, :],
                                 w2t[:, dt, fc, :],
                                 c.h_all[:, fc, s0:s0 + hs, e],
                                 start=(fc == 0), stop=(fc == FC - 1))
        dst = c.eoT[:, half * 4 + dp * 2:half * 4 + (dp + 1) * 2,
                    s0:s0 + hs, e]
        ev = nc.vector.tensor_copy if (e + dp) % 2 else _act_copy(nc)
        ev(out=dst, in_=eo_ps[:])


def _emit_gen(c, half, t0, t1):
    """Pair-of-tiles emit, skewed: the next pair's PE transposes issue
    before the current pair's emit matmuls/evacs; the next pair's eo_blk
    copy issues after them so the in-order DVE queue drains the current
    evacs first."""
    prev = None
    for t in range(t0, t1, 2):
        a = _p3_emitA1(c, half, t)
        if prev is not None:
            _p3_emitB(c, prev)
        _p3_emitA2(c, a)
        if prev is not None:
            yield
        prev = a
    _p3_emitB(c, prev)
    yield


def _p3_emitA1(c, half, t):
    nc = c.nc
    # transpose two eoT tiles -> (s,e)-major blocks
    eb_ps = c.psC.tile([128, 2, 4, 128], BF, tag="Cmt",
                       name=f"ebps{half}_{t}")
    for w in range(2):
        for j in range(4):
            dt = half * 4 + j
            src = c.eoT[:, dt, (t + w) * TILE_S:(t + w + 1) * TILE_S, :]
            nc.tensor.transpose(eb_ps[:, w, j, :],
                                src.rearrange("p s e -> p (s e)"),
                                c.idb_sb[:])
    stt = _Ctx()
    stt.half, stt.t, stt.eb_ps = half, t, eb_ps
    return stt


def _p3_emitA2(c, stt):
    nc = c.nc
    eo_blk = c.rte.tile([128, 2, 4, 128], BF, tag="eoblk",
                        name=f"eob{stt.half}_{stt.t}")
    ev = nc.vector.tensor_copy if stt.t % 4 else _act_copy(nc)
    ev(out=eo_blk[:], in_=stt.eb_ps[:])
    stt.eo_blk = eo_blk


def _p3_emitB(c, stt):
    nc = c.nc
    half, t, eo_blk = stt.half, stt.t, stt.eo_blk
    for w in range(2):
        o_sb = c.outst.tile([128, NBLK, 512], BF, tag="osb",
                            name=f"osb{half}_{t}_{w}")
        for b in range(NBLK):
            r0 = 32 * b
            o_ps = c.psB.tile([128, 512], F32, tag="B",
                              name=f"ops{half}_{t}_{w}_{b}")
            nc.tensor.matmul(o_ps[:], c.wbdT[r0:r0 + 32, t + w, :],
                             eo_blk[r0:r0 + 32, w, :, :],
                             start=True, stop=True, tile_position=(r0, 0))
            ev = nc.vector.tensor_copy if b % 2 else _act_copy(nc)
            ev(out=o_sb[:, b, :], in_=o_ps[:])
        nc.scalar.dma_start(
            c.out_r[:, (t + w) * NBLK:(t + w + 1) * NBLK,
                    half * 512:(half + 1) * 512],
            o_sb[:])


def build_module(s_local, num_devices, reps=1):
    T = s_local * K
    nc = bacc.Bacc("TRN2", target_bir_lowering=False, debug=False,
                   num_devices=num_devices)
    xT = nc.dram_tensor("xT", [D, T], BF, kind="ExternalInput").ap()
    w1 = nc.dram_tensor("w1", [128, E, DC, F], BF, kind="ExternalInput").ap()
    w2 = nc.dram_tensor("w2", [128, E, DC, FC, 128], BF,
                        kind="ExternalInput").ap()
    ctl = nc.dram_tensor("ctl", [128, DC, E], BF, kind="ExternalInput").ap()
    idb = nc.dram_tensor("idb", [128, 128], BF, kind="ExternalInput").ap()
    abd = nc.dram_tensor("abd", [128, 128], BF, kind="ExternalInput").ap()
    msk = nc.dram_tensor("msk", [128, NBLK, E], BF, kind="ExternalInput").ap()
    out = nc.dram_tensor("out", [T, D], BF, kind="ExternalOutput").ap()
    with tile.TileContext(nc) as tc:
        moe_body(tc, xT, w1, w2, ctl, idb, abd, msk, out, s_local, reps=reps)
    nc.compile()
    return nc


def stage_weights(lin1, lin2, controller):
    bf = ml_dtypes.bfloat16
    # [128p, e, dc, f]: element = lin1[e, dc*128+p, f]
    w1h = np.ascontiguousarray(
        lin1.reshape(E, DC, 128, F).transpose(2, 0, 1, 3)).astype(bf)
    # [128p, e, dt, fc, c]: element = lin2[e, fc*128+p, dt*128+c]
    w2h = np.ascontiguousarray(
        lin2.reshape(E, FC, 128, DC, 128).transpose(2, 0, 3, 1, 4)).astype(bf)
    ctlh = np.ascontiguousarray(
        controller.reshape(DC, 128, E).transpose(1, 0, 2)).astype(bf)
    return w1h, w2h, ctlh


def stage_consts():
    bf = ml_dtypes.bfloat16
    idb = np.eye(128, dtype=np.float32).astype(bf)
    # block-diag ones: A[i, j] = 1 iff same s-position (i//32 == j//32)
    abd = np.kron(np.eye(NBLK, dtype=np.float32),
                  np.ones((K, K), np.float32)).astype(bf)
    msk = np.zeros((128, NBLK, E), np.float32)
    for sb in range(NBLK):
        msk[sb * K:(sb + 1) * K, sb, :] = 1.0
    return idb, abd, msk.astype(bf)


def stage_x(xs):
    """xs: (B, s_local, D) fp32 -> xT bf16 (D, T), d-major."""
    s_local = xs.shape[1]
    bf = ml_dtypes.bfloat16
    xT_h = np.ascontiguousarray(
        xs.transpose(2, 1, 0).reshape(D, s_local * K)).astype(bf)
    return xT_h


_MODULE_CACHE = {}


def kernel(x, lin1, lin2, controller):
    from concourse.bass_utils import run_bass_kernel_spmd

    s_local = S // N_CORES
    key = (s_local, N_CORES)
    if key not in _MODULE_CACHE:
        _MODULE_CACHE[key] = build_module(s_local, N_CORES)
    nc = _MODULE_CACHE[key]

    w1h, w2h, ctlh = stage_weights(lin1, lin2, controller)
    idb, abd, msk = stage_consts()
    in_maps = []
    for c in range(N_CORES):
        xT_h = stage_x(x[:, c * s_local:(c + 1) * s_local, :])
        in_maps.append({"xT": xT_h, "w1": w1h, "w2": w2h,
                        "ctl": ctlh, "idb": idb, "abd": abd, "msk": msk})

    res = run_bass_kernel_spmd(nc, in_maps, core_ids=list(range(N_CORES)))
    out_full = np.empty((B, S, D), np.float32)
    for c in range(N_CORES):
        oc = np.asarray(res.results[c]["out"]).astype(np.float32)
        out_full[:, c * s_local:(c + 1) * s_local, :] = (
            oc.reshape(s_local, K, D).transpose(1, 0, 2))
    kernel.last_results = res
    return out_full


# revision 65
# speedup vs baseline: 4.4043x; 2.3850x over previous
"""Trainium2 Bass kernel for nn_MixtureOfTokensLayer.

Math (per sequence position s; B=32 tokens form ONE group of k=32):
  logits = x_s @ controller            (32, 8)
  w      = softmax_k(logits)           (32, 8)
  merged = w.T @ x_s                   (8, 1024)   per-expert token mix
  h      = relu(merged_e @ lin1_e)     (8, 512)
  eo     = h_e @ lin2_e                (8, 1024)
  out_s  = w @ eo                      (32, 1024)

Sharding: data-parallel over S (2048 -> 256 per core, 8 cores), no
collectives. All compute bf16 with fp32 PSUM; output bf16, host upcasts.

Dataflow (single s-chunk of 256; x staged HOST-SIDE as x^T only — the
tok-major copy is rebuilt on-chip by PE transposes, saving 16.8 MB of
HBM traffic per core):
  P1 per 16-s tile (512 tokens, 4 blocks of 128), 1-tile A/B skew so the
  PE never waits on the DVE softmax chain:
    A(t): PE-transpose x^T blocks -> xg blocks (bf16 psum, evac split
      DVE/ACT); logits via narrow matmuls (lhsT = x^T block, rhs = ctl
      chunk, out [tok, 8] accumulated over d-chunks); exp (ACT);
      per-token Z via block-diag-ones matmul; then DVE: 1/Z, normalized
      weights, block-diag wbd.
    B(t-1): merged^T directly (lhsT = xg block chunk, rhs = wbd, out
      [d-chunk, 32] — no merge transposes); wbdT via PE transpose.
  P2 per expert: lin1 (w1 chunk stationary, stream merged^T, N=256),
    relu evac -> h (f-major).
  P3 per d-half: lin2 per expert (w2 stationary, stream h, N=256) ->
    eoT (d-major); then per tile: PE-transpose eoT -> (s,e)-major
    blocks; emit matmuls (lhsT = wbdT row blocks, row-tiled); evac bf16
    alternating DVE/ACT; one output DMA per (half, tile).

Weight DMAs are issued from the sync (SP) ring, interleaved between x
tile loads so transfers land just before their consumers.
"""

import sys

import numpy as np
import ml_dtypes

sys.path.insert(0, "/opt/trn_rl_repo")

import concourse.bass as bass
import concourse.mybir as mybir
import concourse.tile as tile
from concourse import bacc

B, S, D, F, E, K = 32, 2048, 1024, 512, 8, 32
N_CORES = 8
TILE_S = 16              # s-positions per P1 tile
NBLK = 4                 # 128-token blocks per tile
TOK = 128                # tokens per block (4 s * 32 k)
DC = D // 128            # 8 d-chunks
FC = F // 128            # 4 f-chunks
BF = mybir.dt.bfloat16
F32 = mybir.dt.float32
AF = mybir.ActivationFunctionType


class _Ctx:
    pass


def moe_body(tc, xT, w1, w2, ctl, idb, abd, msk, out, s_local, reps=1):
    nc = tc.nc
    import contextlib

    n_tiles = s_local // TILE_S
    cs = s_local

    with (
        tc.tile_pool(name="const", bufs=1) as const,
        tc.tile_pool(name="pers", bufs=1) as pers,
        tc.tile_pool(name="xstream", bufs=3) as xsp,
        tc.tile_pool(name="xgp", bufs=3) as xgp,
        tc.tile_pool(name="route", bufs=2) as rte,
        tc.tile_pool(name="wstream", bufs=2) as wsp,
        tc.tile_pool(name="outst", bufs=3) as outst,
        tc.tile_pool(name="psA", bufs=3, space="PSUM") as psA,
        tc.tile_pool(name="psB", bufs=3, space="PSUM") as psB,
        tc.tile_pool(name="psC", bufs=2, space="PSUM") as psC,
    ):
        c = _Ctx()
        c.tc, c.nc = tc, nc
        c.xsp, c.xgp, c.rte, c.wsp, c.outst = xsp, xgp, rte, wsp, outst
        c.psA, c.psB, c.psC = psA, psB, psC
        c.w1, c.w2 = w1, w2
        c.n_tiles, c.cs = n_tiles, cs

        # ---- constants ----
        c.ctl_sb = const.tile([128, DC, E], BF, name="ctl_sb")
        nc.scalar.dma_start(c.ctl_sb[:], ctl)
        c.idb_sb = const.tile([128, 128], BF, name="idb_sb")
        nc.scalar.dma_start(c.idb_sb[:], idb)
        c.abd_sb = const.tile([128, 128], BF, name="abd_sb")
        nc.scalar.dma_start(c.abd_sb[:], abd)
        c.msk_sb = const.tile([128, NBLK, E], BF, name="msk_sb")
        nc.scalar.dma_start(c.msk_sb[:], msk)

        # ---- persistent intermediates ----
        # merged^T, (s,e)-interleaved columns: [d-part, dc, s, e].
        # eoT ([d-part, dt, s, e]) aliases the same slot later: mT is dead
        # once lin1 has consumed it, so tag-generation rotation reuses it.
        c.pers = pers
        c.mT = pers.tile([128, DC, cs, E], BF, tag="big", bufs=1, name="mT")
        # h, f-major: [f-part, fc, s, e]
        c.h_all = pers.tile([128, FC, cs, E], BF, name="h_all")
        # emit stationary: [(s,e)-part, tile, tok]
        c.wbdT = pers.tile([128, n_tiles, TOK], BF, name="wbdT")

        c.xT_r = xT.rearrange("(dc p) t -> p dc t", p=128)
        c.out_r = out.rearrange("(nb p) d -> p nb d", p=128)

        # w1/w2 stream tiles, DMA'd from the sync ring interleaved with x
        c.w1t = [None] * E
        c.w2t = [None] * (2 * E)

        W1_AT = [48 + 2.9 * i for i in range(E)]
        W2_AT = [68 + 1.5 * i for i in range(2 * E)]

        def load_w1(e, eng=None):
            # hold weight transfers behind the latency-critical x stream
            with tc.tile_wait_until(W1_AT[e] / 1000):
                c.w1t[e] = c.wsp.tile([128, DC, F], BF, tag="w1",
                                      bufs=6, name=f"w1_{e}")
                (eng or nc.sync).dma_start(c.w1t[e][:], c.w1[:, e])

        def load_w2(i):
            half, we = i // E, i % E
            with tc.tile_wait_until(W2_AT[i] / 1000):
                c.w2t[i] = c.wsp.tile([128, 4, FC, 128], BF, tag="w2",
                                      bufs=10, name=f"w2_{half}_{we}")
                nc.sync.dma_start(c.w2t[i][:],
                                  c.w2[:, we, half * 4:(half + 1) * 4])

        rep_ctx = tc.For_i(0, reps, 1) if reps > 1 else contextlib.nullcontext()
        with rep_ctx:
            prev = None  # A-phase state for the 1-tile skew
            for t in range(n_tiles):
                stt = _p1_partA(c, t)
                if prev is not None:
                    _p1_partB(c, prev)
                prev = stt
                # first w1 load rides the very tail of the x stream
                if t == 15:
                    load_w1(0, eng=nc.sync)
            _p1_partB(c, prev)
            # rest of the weights queue behind x on the idle sync ring;
            # pool slot rotation throttles them to just-in-time
            for e in range(1, E):
                load_w1(e)
            for i in range(2 * E):
                load_w2(i)
            for e in range(E):
                _p2_expert(c, e)
            # expert outs, d-major: [d-part, dt, s, e] — reuses mT's slot
            c.eoT = c.pers.tile([128, DC, cs, E], BF, tag="big", bufs=1,
                                name="eoT")
            # lin2 runs in s-halves (s-outer) so each emit batch of 8
            # tiles unblocks after only half the lin2 work of its d-half
            for e in range(E):
                _p3_lin2(c, 0, e, 0)
            gens = [_emit_gen(c, 0, 0, 8), _emit_gen(c, 0, 8, 16),
                    _emit_gen(c, 1, 0, 8)]
            sections = [(0, 1), (1, 0), (1, 1)]
            for g, (half, sh) in zip(gens, sections):
                for e in range(E):
                    _p3_lin2(c, half, e, sh)
                    if e % 2 == 0:
                        next(g, None)
            for g in gens:
                for _ in g:
                    pass
            for _ in _emit_gen(c, 1, 8, 16):
                pass


def _p1_partA(c, t):
    nc = c.nc
    xt = c.xsp.tile([128, DC, TILE_S * K], BF, tag="xT", name=f"xT{t}")
    nc.sync.dma_start(xt[:], c.xT_r[:, :, t * 512:(t + 1) * 512])

    xg = c.xgp.tile([128, NBLK, D], BF, tag="xg", name=f"xg{t}")
    expl = c.rte.tile([128, NBLK, E], BF, tag="expl", name=f"expl{t}")
    # one logits psum tile per 16-s tile: the four per-block accumulation
    # groups run sequentially in the same bank
    lg_ps = c.psB.tile([128, NBLK, E], F32, tag="B", name=f"lgps{t}")
    for b in range(NBLK):
        blk = xt[:, :, b * TOK:(b + 1) * TOK]
        # transposes: x^T block -> xg block (bf16 psum)
        xg_ps = c.psA.tile([128, DC, TOK], BF, tag="A", name=f"xgps{t}_{b}")
        for dc in range(DC):
            nc.tensor.transpose(xg_ps[:, dc, :], blk[:, dc, :], c.idb_sb[:])
        # logits: [tok, 8] accumulated over d-chunks
        for dc in range(DC):
            nc.tensor.matmul(lg_ps[:, b, :], blk[:, dc, :],
                             c.ctl_sb[:, dc, :],
                             start=(dc == 0), stop=(dc == DC - 1))
        if b % 4 == 3:
            ev = _act_copy(nc)
        else:
            ev = nc.vector.tensor_copy
        ev(out=xg[:, b, :], in_=xg_ps[:].rearrange("p dc k -> p (dc k)"))
    nc.scalar.activation(expl[:], lg_ps[:], AF.Exp)

    # per-token Z (replicated over the 32 tokens of each s)
    z_ps = c.psB.tile([128, NBLK, E], F32, tag="B", name=f"zps{t}")
    for b in range(NBLK):
        nc.tensor.matmul(z_ps[:, b, :], c.abd_sb[:], expl[:, b, :],
                         start=True, stop=True)
    zr = c.rte.tile([128, NBLK, E], F32, tag="zr", name=f"zr{t}")
    nc.vector.reciprocal(zr[:], z_ps[:])
    expn = c.rte.tile([128, NBLK, E], BF, tag="expn", name=f"expn{t}")
    nc.vector.tensor_tensor(out=expn[:], in0=expl[:], in1=zr[:],
                            op=mybir.AluOpType.mult)
    wbd = c.rte.tile([TOK, NBLK, NBLK * E], BF, tag="wbd", name=f"wbd{t}")
    for b in range(NBLK):
        nc.vector.tensor_tensor(
            out=wbd[:, b, :],
            in0=expn[:, b, None, :].to_broadcast((TOK, NBLK, E)),
            in1=c.msk_sb[:],
            op=mybir.AluOpType.mult)

    stt = _Ctx()
    stt.t, stt.xg, stt.wbd = t, xg, wbd
    return stt


def _p1_partB(c, stt):
    nc = c.nc
    t, xg, wbd = stt.t, stt.xg, stt.wbd
    wt_ps = c.psC.tile([128, TOK], BF, tag="Cmt", name=f"wtps{t}")
    for p in range(2):  # block pairs share a 2-bank psum tile
        mt_ps = c.psC.tile([128, 2, DC, NBLK * E], F32, tag="Cmt",
                           name=f"mtps{t}_{p}")
        for w in range(2):
            b = p * 2 + w
            # merged^T: lhsT = xg block chunk, rhs = wbd -> [d-chunk, 32]
            for dc in range(DC):
                nc.tensor.matmul(mt_ps[:, w, dc, :],
                                 xg[:, b, dc * 128:(dc + 1) * 128],
                                 wbd[:, b, :], start=True, stop=True)
            r0 = 32 * b
            nc.tensor.transpose(wt_ps[r0:r0 + 32, :], wbd[:, b, :],
                                c.idb_sb[:], tile_position=(0, r0))
        s0 = t * TILE_S + p * 2 * NBLK
        dst = c.mT[:, :, s0:s0 + 2 * NBLK, :]
        ev = nc.vector.tensor_copy if p else _act_copy(nc)
        ev(out=dst.rearrange("p dc (w s) e -> p w dc s e", w=2),
           in_=mt_ps[:].rearrange("p w dc (s e) -> p w dc s e", e=E))
    nc.scalar.copy(out=c.wbdT[:, t, :], in_=wt_ps[:])


def _act_copy(nc):
    def f(out, in_):
        return nc.scalar.copy(out=out, in_=in_)
    return f


def _p2_expert(c, e):
    nc = c.nc
    w1t = c.w1t[e]
    for fp in range(2):  # ft pairs
        h_ps = c.psA.tile([128, 2, c.cs], F32, tag="A", name=f"hps{e}_{fp}")
        for j in range(2):
            ft = fp * 2 + j
            for dc in range(DC):
                nc.tensor.matmul(h_ps[:, j, :],
                                 w1t[:, dc, ft * 128:(ft + 1) * 128],
                                 c.mT[:, dc, :, e],
                                 start=(dc == 0), stop=(dc == DC - 1))
        nc.scalar.activation(c.h_all[:, fp * 2:(fp + 1) * 2, :, e], h_ps[:],
                             AF.Relu)


def _p3_lin2(c, half, e, sh, nq=2):
    nc = c.nc
    w2t = c.w2t[half * E + e]
    hs = c.cs // nq
    s0 = sh * hs
    for dp in range(2):  # dt pairs within the half
        eo_ps = c.psA.tile([128, 2, hs], F32, tag="A",
                           name=f"eops{half}_{e}_{sh}_{dp}")
        for j in range(2):
            dt = dp * 2 + j
            for fc in range(FC):
                nc.tensor.matmul(eo_ps[:, j, :],
                                 w2t[:, dt, fc, :],
                                 c.h_all[:, fc, s0:s0 + hs, e],
                                 start=(fc == 0), stop=(fc == FC - 1))
        dst = c.eoT[:, half * 4 + dp * 2:half * 4 + (dp + 1) * 2,
                    s0:s0 + hs, e]
        nc.vector.tensor_copy(out=dst, in_=eo_ps[:])


def _emit_gen(c, half, t0, t1):
    """Pair-of-tiles emit, skewed: the next pair's PE transposes issue
    before the current pair's emit matmuls/evacs; the next pair's eo_blk
    copy issues after them so the in-order DVE queue drains the current
    evacs first."""
    prev = None
    for t in range(t0, t1, 2):
        a = _p3_emitA1(c, half, t)
        if prev is not None:
            _p3_emitB(c, prev)
        _p3_emitA2(c, a)
        if prev is not None:
            yield
        prev = a
    _p3_emitB(c, prev)
    yield


def _p3_emitA1(c, half, t):
    nc = c.nc
    # transpose two eoT tiles -> (s,e)-major blocks
    eb_ps = c.psC.tile([128, 2, 4, 128], BF, tag="Cmt",
                       name=f"ebps{half}_{t}")
    for w in range(2):
        for j in range(4):
            dt = half * 4 + j
            src = c.eoT[:, dt, (t + w) * TILE_S:(t + w + 1) * TILE_S, :]
            nc.tensor.transpose(eb_ps[:, w, j, :],
                                src.rearrange("p s e -> p (s e)"),
                                c.idb_sb[:])
    stt = _Ctx()
    stt.half, stt.t, stt.eb_ps = half, t, eb_ps
    return stt


def _p3_emitA2(c, stt):
    nc = c.nc
    eo_blk = c.rte.tile([128, 2, 4, 128], BF, tag="eoblk",
                        name=f"eob{stt.half}_{stt.t}")
    ev = nc.vector.tensor_copy if stt.t % 4 else _act_copy(nc)
    ev(out=eo_blk[:], in_=stt.eb_ps[:])
    stt.eo_blk = eo_blk


def _p3_emitB(c, stt):
    nc = c.nc
    half, t, eo_blk = stt.half, stt.t, stt.eo_blk
    for w in range(2):
        o_sb = c.outst.tile([128, NBLK, 512], BF, tag="osb",
                            name=f"osb{half}_{t}_{w}")
        for b in range(NBLK):
            r0 = 32 * b
            pool, tag = (c.psC, "Cmt") if b == 3 else (c.psB, "B")
            o_ps = pool.tile([128, 512], F32, tag=tag,
                             name=f"ops{half}_{t}_{w}_{b}")
            nc.tensor.matmul(o_ps[:], c.wbdT[r0:r0 + 32, t + w, :],
                             eo_blk[r0:r0 + 32, w, :, :],
                             start=True, stop=True, tile_position=(r0, 0))
            ev = nc.vector.tensor_copy if b % 2 else _act_copy(nc)
            ev(out=o_sb[:, b, :], in_=o_ps[:])
        nc.scalar.dma_start(
            c.out_r[:, (t + w) * NBLK:(t + w + 1) * NBLK,
                    half * 512:(half + 1) * 512],
            o_sb[:])


def build_module(s_local, num_devices, reps=1):
    T = s_local * K
    nc = bacc.Bacc("TRN2", target_bir_lowering=False, debug=False,
                   num_devices=num_devices)
    xT = nc.dram_tensor("xT", [D, T], BF, kind="ExternalInput").ap()
    w1 = nc.dram_tensor("w1", [128, E, DC, F], BF, kind="ExternalInput").ap()
    w2 = nc.dram_tensor("w2", [128, E, DC, FC, 128], BF,
                        kind="ExternalInput").ap()
    ctl = nc.dram_tensor("ctl", [128, DC, E], BF, kind="ExternalInput").ap()
    idb = nc.dram_tensor("idb", [128, 128], BF, kind="ExternalInput").ap()
    abd = nc.dram_tensor("abd", [128, 128], BF, kind="ExternalInput").ap()
    msk = nc.dram_tensor("msk", [128, NBLK, E], BF, kind="ExternalInput").ap()
    out = nc.dram_tensor("out", [T, D], BF, kind="ExternalOutput").ap()
    with tile.TileContext(nc) as tc:
        moe_body(tc, xT, w1, w2, ctl, idb, abd, msk, out, s_local, reps=reps)
    nc.compile()
    return nc


def stage_weights(lin1, lin2, controller):
    bf = ml_dtypes.bfloat16
    # [128p, e, dc, f]: element = lin1[e, dc*128+p, f]
    w1h = np.ascontiguousarray(
        lin1.reshape(E, DC, 128, F).transpose(2, 0, 1, 3)).astype(bf)
    # [128p, e, dt, fc, c]: element = lin2[e, fc*128+p, dt*128+c]
    w2h = np.ascontiguousarray(
        lin2.reshape(E, FC, 128, DC, 128).transpose(2, 0, 3, 1, 4)).astype(bf)
    ctlh = np.ascontiguousarray(
        controller.reshape(DC, 128, E).transpose(1, 0, 2)).astype(bf)
    return w1h, w2h, ctlh


def stage_consts():
    bf = ml_dtypes.bfloat16
    idb = np.eye(128, dtype=np.float32).astype(bf)
    # block-diag ones: A[i, j] = 1 iff same s-position (i//32 == j//32)
    abd = np.kron(np.eye(NBLK, dtype=np.float32),
                  np.ones((K, K), np.float32)).astype(bf)
    msk = np.zeros((128, NBLK, E), np.float32)
    for sb in range(NBLK):
        msk[sb * K:(sb + 1) * K, sb, :] = 1.0
    return idb, abd, msk.astype(bf)


def stage_x(xs):
    """xs: (B, s_local, D) fp32 -> xT bf16 (D, T), d-major."""
    s_local = xs.shape[1]
    bf = ml_dtypes.bfloat16
    xT_h = np.ascontiguousarray(
        xs.transpose(2, 1, 0).reshape(D, s_local * K)).astype(bf)
    return xT_h


_MODULE_CACHE = {}


def kernel(x, lin1, lin2, controller):
    from concourse.bass_utils import run_bass_kernel_spmd

    s_local = S // N_CORES
    key = (s_local, N_CORES)
    if key not in _MODULE_CACHE:
        _MODULE_CACHE[key] = build_module(s_local, N_CORES)
    nc = _MODULE_CACHE[key]

    w1h, w2h, ctlh = stage_weights(lin1, lin2, controller)
    idb, abd, msk = stage_consts()
    in_maps = []
    for c in range(N_CORES):
        xT_h = stage_x(x[:, c * s_local:(c + 1) * s_local, :])
        in_maps.append({"xT": xT_h, "w1": w1h, "w2": w2h,
                        "ctl": ctlh, "idb": idb, "abd": abd, "msk": msk})

    res = run_bass_kernel_spmd(nc, in_maps, core_ids=list(range(N_CORES)))
    out_full = np.empty((B, S, D), np.float32)
    for c in range(N_CORES):
        oc = np.asarray(res.results[c]["out"]).astype(np.float32)
        out_full[:, c * s_local:(c + 1) * s_local, :] = (
            oc.reshape(s_local, K, D).transpose(1, 0, 2))
    kernel.last_results = res
    return out_full


# revision 89
# speedup vs baseline: 11.7117x; 2.6591x over previous
"""Trainium2 Bass kernel for nn_MixtureOfTokensLayer.

Math (per sequence position s; B=32 tokens form ONE group of k=32):
  logits = x_s @ controller            (32, 8)
  w      = softmax_k(logits)           (32, 8)
  merged = w.T @ x_s                   (8, 1024)   per-expert token mix
  h      = relu(merged_e @ lin1_e)     (8, 512)
  eo     = h_e @ lin2_e                (8, 1024)
  out_s  = w @ eo                      (32, 1024)

Sharding: data-parallel over S (2048 -> 256 per core, 8 cores), no
collectives. All compute bf16 with fp32 PSUM; output bf16, host upcasts.

Dataflow (single s-chunk of 256; x staged HOST-SIDE as x^T only — the
tok-major copy is rebuilt on-chip by PE transposes, saving 16.8 MB of
HBM traffic per core):
  P1 per 16-s tile (512 tokens, 4 blocks of 128), 1-tile A/B skew so the
  PE never waits on the DVE softmax chain:
    A(t): PE-transpose x^T blocks -> xg blocks (bf16 psum, evac split
      DVE/ACT); logits via narrow matmuls (lhsT = x^T block, rhs = ctl
      chunk, out [tok, 8] accumulated over d-chunks); exp (ACT);
      per-token Z via block-diag-ones matmul; then DVE: 1/Z, normalized
      weights, block-diag wbd.
    B(t-1): merged^T directly (lhsT = xg block chunk, rhs = wbd, out
      [d-chunk, 32] — no merge transposes); wbdT via PE transpose.
  P2 per expert: lin1 (w1 chunk stationary, stream merged^T, N=256),
    relu evac -> h (f-major).
  P3 per d-half: lin2 per expert (w2 stationary, stream h, N=256) ->
    eoT (d-major); then per tile: PE-transpose eoT -> (s,e)-major
    blocks; emit matmuls (lhsT = wbdT row blocks, row-tiled); evac bf16
    alternating DVE/ACT; one output DMA per (half, tile).

Weight DMAs are issued from the sync (SP) ring, interleaved between x
tile loads so transfers land just before their consumers.
"""

import sys

import numpy as np
import ml_dtypes

sys.path.insert(0, "/opt/trn_rl_repo")

import concourse.bass as bass
import concourse.mybir as mybir
import concourse.tile as tile
from concourse import bacc

B, S, D, F, E, K = 32, 2048, 1024, 512, 8, 32
N_CORES = 8
TILE_S = 16              # s-positions per P1 tile
NBLK = 4                 # 128-token blocks per tile
TOK = 128                # tokens per block (4 s * 32 k)
DC = D // 128            # 8 d-chunks
FC = F // 128            # 4 f-chunks
BF = mybir.dt.bfloat16
F32 = mybir.dt.float32
AF = mybir.ActivationFunctionType


class _Ctx:
    pass


def moe_body(tc, xT, w1, w2, ctl, idb, abd, msk, out, s_local, reps=1):
    nc = tc.nc
    import contextlib

    n_tiles = s_local // TILE_S
    cs = s_local

    with (
        tc.tile_pool(name="const", bufs=1) as const,
        tc.tile_pool(name="pers", bufs=1) as pers,
        tc.tile_pool(name="xstream", bufs=3) as xsp,
        tc.tile_pool(name="xgp", bufs=3) as xgp,
        tc.tile_pool(name="route", bufs=2) as rte,
        tc.tile_pool(name="wstream", bufs=2) as wsp,
        tc.tile_pool(name="outst", bufs=3) as outst,
        tc.tile_pool(name="psA", bufs=3, space="PSUM") as psA,
        tc.tile_pool(name="psB", bufs=3, space="PSUM") as psB,
        tc.tile_pool(name="psC", bufs=2, space="PSUM") as psC,
    ):
        c = _Ctx()
        c.tc, c.nc = tc, nc
        c.xsp, c.xgp, c.rte, c.wsp, c.outst = xsp, xgp, rte, wsp, outst
        c.psA, c.psB, c.psC = psA, psB, psC
        c.w1, c.w2 = w1, w2
        c.n_tiles, c.cs = n_tiles, cs

        # ---- constants ----
        c.ctl_sb = const.tile([128, DC, E], BF, name="ctl_sb")
        nc.scalar.dma_start(c.ctl_sb[:], ctl)
        c.idb_sb = const.tile([128, 128], BF, name="idb_sb")
        nc.scalar.dma_start(c.idb_sb[:], idb)
        c.abd_sb = const.tile([128, 128], BF, name="abd_sb")
        nc.scalar.dma_start(c.abd_sb[:], abd)
        c.msk_sb = const.tile([128, NBLK, E], BF, name="msk_sb")
        nc.scalar.dma_start(c.msk_sb[:], msk)

        # ---- persistent intermediates ----
        # merged^T, (s,e)-interleaved columns: [d-part, dc, s, e].
        # eoT ([d-part, dt, s, e]) aliases the same slot later: mT is dead
        # once lin1 has consumed it, so tag-generation rotation reuses it.
        c.pers = pers
        c.mT = pers.tile([128, DC, cs, E], BF, tag="big", bufs=1, name="mT")
        # h, f-major: [f-part, fc, s, e]
        c.h_all = pers.tile([128, FC, cs, E], BF, name="h_all")
        # emit stationary: [(s,e)-part, tile, tok]
        c.wbdT = pers.tile([128, n_tiles, TOK], BF, name="wbdT")

        c.xT_r = xT.rearrange("(dc p) t -> p dc t", p=128)
        c.out_r = out.rearrange("(nb p) d -> p nb d", p=128)

        # w1/w2 stream tiles, DMA'd from the sync ring interleaved with x
        c.w1t = [None] * E
        c.w2t = [None] * (2 * E)

        W1_AT = [46 + 2.9 * i for i in range(E)]
        W2_AT = [65 + 1.5 * i for i in range(2 * E)]

        def load_w1(e, eng=None):
            # hold weight transfers behind the latency-critical x stream
            with tc.tile_wait_until(W1_AT[e] / 1000):
                c.w1t[e] = c.wsp.tile([128, DC, F], BF, tag="w1",
                                      bufs=6, name=f"w1_{e}")
                (eng or nc.sync).dma_start(c.w1t[e][:], c.w1[:, e])

        def load_w2(i):
            half, we = i // E, i % E
            with tc.tile_wait_until(W2_AT[i] / 1000):
                c.w2t[i] = c.wsp.tile([128, 4, FC, 128], BF, tag="w2",
                                      bufs=10, name=f"w2_{half}_{we}")
                nc.sync.dma_start(c.w2t[i][:],
                                  c.w2[:, we, half * 4:(half + 1) * 4])

        rep_ctx = tc.For_i(0, reps, 1) if reps > 1 else contextlib.nullcontext()
        with rep_ctx:
            prev = None  # A-phase state for the 1-tile skew
            for t in range(n_tiles):
                stt = _p1_partA(c, t)
                if prev is not None:
                    _p1_partB(c, prev)
                prev = stt
                # first w1 load rides the very tail of the x stream
                if t == 15:
                    load_w1(0, eng=nc.sync)
            _p1_partB(c, prev)
            # rest of the weights queue behind x on the idle sync ring;
            # pool slot rotation throttles them to just-in-time
            for e in range(1, E):
                load_w1(e)
            for i in range(2 * E):
                load_w2(i)
            for e in range(E):
                _p2_expert(c, e)
            # expert outs, d-major: [d-part, dt, s, e] — reuses mT's slot
            c.eoT = c.pers.tile([128, DC, cs, E], BF, tag="big", bufs=1,
                                name="eoT")
            # lin2 runs in s-halves (s-outer) so each emit batch of 8
            # tiles unblocks after only half the lin2 work of its d-half
            for e in range(E):
                _p3_lin2(c, 0, e, 0)
            gens = [_emit_gen(c, 0, 0, 8), _emit_gen(c, 0, 8, 16),
                    _emit_gen(c, 1, 0, 8)]
            sections = [(0, 1), (1, 0), (1, 1)]
            for g, (half, sh) in zip(gens, sections):
                for e in range(E):
                    _p3_lin2(c, half, e, sh)
                    if e % 2 == 0:
                        next(g, None)
            for g in gens:
                for _ in g:
                    pass
            for _ in _emit_gen(c, 1, 8, 16):
                pass


def _p1_partA(c, t):
    nc = c.nc
    xt = c.xsp.tile([128, DC, TILE_S * K], BF, tag="xT", name=f"xT{t}")
    nc.sync.dma_start(xt[:], c.xT_r[:, :, t * 512:(t + 1) * 512])

    xg = c.xgp.tile([128, NBLK, D], BF, tag="xg", name=f"xg{t}")
    expl = c.rte.tile([128, NBLK, E], BF, tag="expl", name=f"expl{t}")
    # one logits psum tile per 16-s tile: the four per-block accumulation
    # groups run sequentially in the same bank
    lg_ps = c.psB.tile([128, NBLK, E], F32, tag="B", name=f"lgps{t}")
    for b in range(NBLK):
        blk = xt[:, :, b * TOK:(b + 1) * TOK]
        # transposes: x^T block -> xg block (bf16 psum)
        xg_ps = c.psA.tile([128, DC, TOK], BF, tag="A", name=f"xgps{t}_{b}")
        for dc in range(DC):
            nc.tensor.transpose(xg_ps[:, dc, :], blk[:, dc, :], c.idb_sb[:])
        # logits: [tok, 8] accumulated over d-chunks
        for dc in range(DC):
            nc.tensor.matmul(lg_ps[:, b, :], blk[:, dc, :],
                             c.ctl_sb[:, dc, :],
                             start=(dc == 0), stop=(dc == DC - 1))
        if b % 4 == 0:
            ev = _act_copy(nc)
        else:
            ev = nc.vector.tensor_copy
        ev(out=xg[:, b, :], in_=xg_ps[:].rearrange("p dc k -> p (dc k)"))
    nc.scalar.activation(expl[:], lg_ps[:], AF.Exp)

    # per-token Z (replicated over the 32 tokens of each s)
    z_ps = c.psB.tile([128, NBLK, E], F32, tag="B", name=f"zps{t}")
    for b in range(NBLK):
        nc.tensor.matmul(z_ps[:, b, :], c.abd_sb[:], expl[:, b, :],
                         start=True, stop=True)
    zr = c.rte.tile([128, NBLK, E], F32, tag="zr", name=f"zr{t}")
    nc.vector.reciprocal(zr[:], z_ps[:])
    expn = c.rte.tile([128, NBLK, E], BF, tag="expn", name=f"expn{t}")
    nc.vector.tensor_tensor(out=expn[:], in0=expl[:], in1=zr[:],
                            op=mybir.AluOpType.mult)
    wbd = c.rte.tile([TOK, NBLK, NBLK * E], BF, tag="wbd", name=f"wbd{t}")
    for b in range(NBLK):
        nc.gpsimd.tensor_tensor(
            out=wbd[:, b, :],
            in0=expn[:, b, None, :].to_broadcast((TOK, NBLK, E)),
            in1=c.msk_sb[:],
            op=mybir.AluOpType.mult)

    stt = _Ctx()
    stt.t, stt.xg, stt.wbd = t, xg, wbd
    return stt


def _p1_partB(c, stt):
    nc = c.nc
    t, xg, wbd = stt.t, stt.xg, stt.wbd
    wt_ps = c.psC.tile([128, TOK], BF, tag="Cmt", name=f"wtps{t}")
    for p in range(2):  # block pairs share a 2-bank psum tile
        mt_ps = c.psC.tile([128, 2, DC, NBLK * E], F32, tag="Cmt",
                           name=f"mtps{t}_{p}")
        for w in range(2):
            b = p * 2 + w
            # merged^T: lhsT = xg block chunk, rhs = wbd -> [d-chunk, 32]
            for dc in range(DC):
                nc.tensor.matmul(mt_ps[:, w, dc, :],
                                 xg[:, b, dc * 128:(dc + 1) * 128],
                                 wbd[:, b, :], start=True, stop=True)
            r0 = 32 * b
            nc.tensor.transpose(wt_ps[r0:r0 + 32, :], wbd[:, b, :],
                                c.idb_sb[:], tile_position=(0, r0))
        s0 = t * TILE_S + p * 2 * NBLK
        dst = c.mT[:, :, s0:s0 + 2 * NBLK, :]
        ev = _act_copy(nc) if p else nc.vector.tensor_copy
        ev(out=dst.rearrange("p dc (w s) e -> p w dc s e", w=2),
           in_=mt_ps[:].rearrange("p w dc (s e) -> p w dc s e", e=E))
    nc.vector.tensor_copy(out=c.wbdT[:, t, :], in_=wt_ps[:])


def _act_copy(nc):
    def f(out, in_):
        return nc.scalar.copy(out=out, in_=in_)
    return f


def _p2_expert(c, e):
    nc = c.nc
    w1t = c.w1t[e]
    for fp in range(2):  # ft pairs
        h_ps = c.psA.tile([128, 2, c.cs], F32, tag="A", name=f"hps{e}_{fp}")
        for j in range(2):
            ft = fp * 2 + j
            for dc in range(DC):
                nc.tensor.matmul(h_ps[:, j, :],
                                 w1t[:, dc, ft * 128:(ft + 1) * 128],
                                 c.mT[:, dc, :, e],
                                 start=(dc == 0), stop=(dc == DC - 1))
        nc.scalar.activation(c.h_all[:, fp * 2:(fp + 1) * 2, :, e], h_ps[:],
                             AF.Relu)


def _p3_lin2(c, half, e, sh, nq=2):
    nc = c.nc
    w2t = c.w2t[half * E + e]
    hs = c.cs // nq
    s0 = sh * hs
    for dp in range(2):  # dt pairs within the half
        eo_ps = c.psA.tile([128, 2, hs], F32, tag="A",
                           name=f"eops{half}_{e}_{sh}_{dp}")
        for j in range(2):
            dt = dp * 2 + j
            for fc in range(FC):
                nc.tensor.matmul(eo_ps[:, j, :],
                                 w2t[:, dt, fc, :],
                                 c.h_all[:, fc, s0:s0 + hs, e],
                                 start=(fc == 0), stop=(fc == FC - 1))
        dst = c.eoT[:, half * 4 + dp * 2:half * 4 + (dp + 1) * 2,
                    s0:s0 + hs, e]
        nc.vector.tensor_copy(out=dst, in_=eo_ps[:])


def _emit_gen(c, half, t0, t1):
    """Pair-of-tiles emit, skewed: the next pair's PE transposes issue
    before the current pair's emit matmuls/evacs; the next pair's eo_blk
    copy issues after them so the in-order DVE queue drains the current
    evacs first."""
    prev = None
    for t in range(t0, t1, 2):
        a = _p3_emitA1(c, half, t)
        if prev is not None:
            _p3_emitB(c, prev)
        _p3_emitA2(c, a)
        if prev is not None:
            yield
        prev = a
    _p3_emitB(c, prev)
    yield


def _p3_emitA1(c, half, t):
    nc = c.nc
    # transpose two eoT tiles -> (s,e)-major blocks
    eb_ps = c.psC.tile([128, 2, 4, 128], BF, tag="Cmt",
                       name=f"ebps{half}_{t}")
    for w in range(2):
        for j in range(4):
            dt = half * 4 + j
            src = c.eoT[:, dt, (t + w) * TILE_S:(t + w + 1) * TILE_S, :]
            nc.tensor.transpose(eb_ps[:, w, j, :],
                                src.rearrange("p s e -> p (s e)"),
                                c.idb_sb[:])
    stt = _Ctx()
    stt.half, stt.t, stt.eb_ps = half, t, eb_ps
    return stt


def _p3_emitA2(c, stt):
    nc = c.nc
    eo_blk = c.rte.tile([128, 2, 4, 128], BF, tag="eoblk",
                        name=f"eob{stt.half}_{stt.t}")
    ev = nc.vector.tensor_copy if stt.t % 4 else _act_copy(nc)
    ev(out=eo_blk[:], in_=stt.eb_ps[:])
    stt.eo_blk = eo_blk


def _p3_emitB(c, stt):
    nc = c.nc
    half, t, eo_blk = stt.half, stt.t, stt.eo_blk
    for w in range(2):
        o_sb = c.outst.tile([128, NBLK, 512], BF, tag="osb",
                            name=f"osb{half}_{t}_{w}")
        for b in range(NBLK):
            r0 = 32 * b
            pool, tag = (c.psC, "Cmt") if b == 3 else (c.psB, "B")
            o_ps = pool.tile([128, 512], F32, tag=tag,
                             name=f"ops{half}_{t}_{w}_{b}")
            nc.tensor.matmul(o_ps[:], c.wbdT[r0:r0 + 32, t + w, :],
                             eo_blk[r0:r0 + 32, w, :, :],
                             start=True, stop=True, tile_position=(r0, 0))
            ev = _act_copy(nc) if b % 2 else nc.vector.tensor_copy
            ev(out=o_sb[:, b, :], in_=o_ps[:])
        nc.scalar.dma_start(
            c.out_r[:, (t + w) * NBLK:(t + w + 1) * NBLK,
                    half * 512:(half + 1) * 512],
            o_sb[:])


def build_module(s_local, num_devices, reps=1):
    T = s_local * K
    nc = bacc.Bacc("TRN2", target_bir_lowering=False, debug=False,
                   num_devices=num_devices)
    xT = nc.dram_tensor("xT", [D, T], BF, kind="ExternalInput").ap()
    w1 = nc.dram_tensor("w1", [128, E, DC, F], BF, kind="ExternalInput").ap()
    w2 = nc.dram_tensor("w2", [128, E, DC, FC, 128], BF,
                        kind="ExternalInput").ap()
    ctl = nc.dram_tensor("ctl", [128, DC, E], BF, kind="ExternalInput").ap()
    idb = nc.dram_tensor("idb", [128, 128], BF, kind="ExternalInput").ap()
    abd = nc.dram_tensor("abd", [128, 128], BF, kind="ExternalInput").ap()
    msk = nc.dram_tensor("msk", [128, NBLK, E], BF, kind="ExternalInput").ap()
    out = nc.dram_tensor("out", [T, D], BF, kind="ExternalOutput").ap()
    with tile.TileContext(nc) as tc:
        moe_body(tc, xT, w1, w2, ctl, idb, abd, msk, out, s_local, reps=reps)
    nc.compile()
    return nc


def stage_weights(lin1, lin2, controller):
    bf = ml_dtypes.bfloat16
    # [128p, e, dc, f]: element = lin1[e, dc*128+p, f]
    w1h = np.ascontiguousarray(
        lin1.reshape(E, DC, 128, F).transpose(2, 0, 1, 3)).astype(bf)
    # [128p, e, dt, fc, c]: element = lin2[e, fc*128+p, dt*128+c]
    w2h = np.ascontiguousarray(
        lin2.reshape(E, FC, 128, DC, 128).transpose(2, 0, 3, 1, 4)).astype(bf)
    ctlh = np.ascontiguousarray(
        controller.reshape(DC, 128, E).transpose(1, 0, 2)).astype(bf)
    return w1h, w2h, ctlh


def stage_consts():
    bf = ml_dtypes.bfloat16
    idb = np.eye(128, dtype=np.float32).astype(bf)
    # block-diag ones: A[i, j] = 1 iff same s-position (i//32 == j//32)
    abd = np.kron(np.eye(NBLK, dtype=np.float32),
                  np.ones((K, K), np.float32)).astype(bf)
    msk = np.zeros((128, NBLK, E), np.float32)
    for sb in range(NBLK):
        msk[sb * K:(sb + 1) * K, sb, :] = 1.0
    return idb, abd, msk.astype(bf)


def stage_x(xs):
    """xs: (B, s_local, D) fp32 -> xT bf16 (D, T), d-major."""
    s_local = xs.shape[1]
    bf = ml_dtypes.bfloat16
    xT_h = np.ascontiguousarray(
        xs.transpose(2, 1, 0).reshape(D, s_local * K)).astype(bf)
    return xT_h


_MODULE_CACHE = {}


def kernel(x, lin1, lin2, controller):
    from concourse.bass_utils import run_bass_kernel_spmd

    s_local = S // N_CORES
    key = (s_local, N_CORES)
    if key not in _MODULE_CACHE:
        _MODULE_CACHE[key] = build_module(s_local, N_CORES)
    nc = _MODULE_CACHE[key]

    w1h, w2h, ctlh = stage_weights(lin1, lin2, controller)
    idb, abd, msk = stage_consts()
    in_maps = []
    for c in range(N_CORES):
        xT_h = stage_x(x[:, c * s_local:(c + 1) * s_local, :])
        in_maps.append({"xT": xT_h, "w1": w1h, "w2": w2h,
                        "ctl": ctlh, "idb": idb, "abd": abd, "msk": msk})

    res = run_bass_kernel_spmd(nc, in_maps, core_ids=list(range(N_CORES)))
    out_full = np.empty((B, S, D), np.float32)
    for c in range(N_CORES):
        oc = np.asarray(res.results[c]["out"]).astype(np.float32)
        out_full[:, c * s_local:(c + 1) * s_local, :] = (
            oc.reshape(s_local, K, D).transpose(1, 0, 2))
    kernel.last_results = res
    return out_full
